# revision 1
# baseline (speedup 1.0000x reference)
"""YOLOv2-style loss (nn_CostYoloV2) on 8 Trainium2 NeuronCores.

Sharding:
  * per-(batch,truth) losses (obj / class / coords): data-parallel over batch,
    8 batches per core.  The per-cell channel gather x[b, :, j, i] is done
    on-chip with a gpsimd indirect_copy from the natural [ch, cell] layout,
    then PE-transposed into [bt, ch] tiles for the vector-engine loss math.
  * the noobj term and the coord warmup term only involve batch 63 - those are
    sharded over the 1024 spatial cells (128 cells per core, all 5 anchors).
  * scalar partials are returned per core and combined on the host (the
    "all-reduce" of the loss terms).

Host prep is limited to O(B*T) index/truth-derived tensors (cell indices,
one-hots, exp(w)*anchor etc.) and slicing the inputs per core.
"""
import numpy as np
from contextlib import ExitStack

import concourse.bass as bass
import concourse.bacc as bacc
import concourse.mybir as mybir
import concourse.tile as tile
from concourse.bass_utils import run_bass_kernel_spmd
from concourse.masks import make_identity

F32 = mybir.dt.float32
U16 = mybir.dt.uint16
Alu = mybir.AluOpType
Ax = mybir.AxisListType

B, NUM, CLASSES, AL = 64, 5, 20, 25
H = W = 32
HW = H * W
T = 50
NCORES = 8
BL = B // NCORES      # local batches per core
TILES = BL // 2       # bt tiles of 2 batches x 50 truths
ROWS = 2 * T          # 100 partitions per bt tile
CELLS = HW // NCORES  # 128 noobj cells per core

_CACHED = {}


def _build_program():
    nc = bacc.Bacc()
    xin = nc.declare_dram_parameter("xin", [BL, NUM * AL, HW], F32, isOutput=False)
    gidx = nc.declare_dram_parameter("gidx", [128, BL * 4], U16, isOutput=False)
    tv = nc.declare_dram_parameter("tv", [TILES, ROWS, 28], F32, isOutput=False)
    p63 = nc.declare_dram_parameter("p63", [128, 20], F32, isOutput=False)
    o63sq = nc.declare_dram_parameter("o63sq", [128, 1], F32, isOutput=False)
    tb63 = nc.declare_dram_parameter("tb63", [128, 250], F32, isOutput=False)
    obt = nc.declare_dram_parameter("obt", [TILES, ROWS, 3], F32, isOutput=True)
    onoobj = nc.declare_dram_parameter("onoobj", [128, 1], F32, isOutput=True)
    owarm = nc.declare_dram_parameter("owarm", [128, 2], F32, isOutput=True)

    with tile.TileContext(nc) as tc, ExitStack() as ctx:
        const = ctx.enter_context(tc.tile_pool(name="const", bufs=1))
        xpool = ctx.enter_context(tc.tile_pool(name="xp", bufs=8))
        work = ctx.enter_context(tc.tile_pool(name="wk", bufs=4))
        psum = ctx.enter_context(tc.tile_pool(name="ps", bufs=4, space="PSUM"))

        ident = const.tile([128, 128], F32)
        make_identity(nc, ident[:])
        gx = const.tile([128, BL * 4], U16)
        nc.sync.dma_start(gx[:], gidx[:])
        tb = const.tile([128, 250], F32)
        nc.sync.dma_start(tb[:], tb63[:])
        pp = const.tile([128, 20], F32)
        nc.sync.dma_start(pp[:], p63[:])
        oq = const.tile([128, 1], F32)
        nc.sync.dma_start(oq[:], o63sq[:])

        # ---------------- noobj (batch 63, this core's 128 cells) -------------
        pv = pp[:].rearrange("p (n c) -> p n c", c=4)
        xc, yc, wc, hc = pv[:, :, 0], pv[:, :, 1], pv[:, :, 2], pv[:, :, 3]
        corn = const.tile([128, 25], F32)  # al|au|ar|ad|hap blocks of 5
        al, au = corn[:, 0:5], corn[:, 5:10]
        ar, ad = corn[:, 10:15], corn[:, 15:20]
        hap = corn[:, 20:25]
        nc.vector.scalar_tensor_tensor(al, wc, -0.5, xc, Alu.mult, Alu.add)
        nc.vector.scalar_tensor_tensor(au, hc, -0.5, yc, Alu.mult, Alu.add)
        nc.vector.scalar_tensor_tensor(ar, wc, 0.5, xc, Alu.mult, Alu.add)
        nc.vector.scalar_tensor_tensor(ad, hc, 0.5, yc, Alu.mult, Alu.add)
        nc.vector.scalar_tensor_tensor(hap, wc, 0.5, hc, Alu.mult, Alu.mult)

        def bc_p(apv):  # [128, 5] -> [128, 5, 50]
            return apv.broadcast_to([128, 5, T])

        def bc_t(col):  # tb block [128, 50] -> [128, 5, 50]
            return tb[:, col * T:(col + 1) * T].rearrange(
                "p (o f) -> p o f", o=1).broadcast_to([128, 5, T])

        def w3(tl):  # [128, 250] tile -> [128, 5, 50] view
            return tl[:].rearrange("p (n t) -> p n t", t=T)

        m1 = const.tile([128, 5 * T], F32)
        m2 = const.tile([128, 5 * T], F32)
        iw = const.tile([128, 5 * T], F32)
        ih = const.tile([128, 5 * T], F32)
        nc.vector.tensor_tensor(w3(m1), bc_t(2), bc_p(ar), Alu.min)
        nc.vector.tensor_tensor(w3(m2), bc_t(0), bc_p(al), Alu.max)
        nc.vector.tensor_tensor(iw[:], m1[:], m2[:], Alu.subtract)
        nc.vector.tensor_scalar(iw[:], iw[:], 0.0, None, Alu.max)
        nc.vector.tensor_tensor(w3(m1), bc_t(3), bc_p(ad), Alu.min)
        nc.vector.tensor_tensor(w3(m2), bc_t(1), bc_p(au), Alu.max)
        nc.vector.tensor_tensor(ih[:], m1[:], m2[:], Alu.subtract)
        nc.vector.tensor_scalar(ih[:], ih[:], 0.0, None, Alu.max)
        nc.vector.tensor_tensor(iw[:], iw[:], ih[:], Alu.mult)   # inter
        nc.vector.tensor_scalar(iw[:], iw[:], 1.5, None, Alu.mult)
        nc.vector.tensor_tensor(w3(iw), w3(iw), bc_p(hap), Alu.subtract)
        nc.vector.tensor_tensor(w3(m1), w3(iw), bc_t(4), Alu.is_gt)  # mask
        anyt = const.tile([128, 5], F32)
        nc.vector.tensor_reduce(anyt[:], w3(m1), Ax.X, Alu.max)
        smk = const.tile([128, 1], F32)
        nc.vector.tensor_reduce(smk[:], anyt[:], Ax.X, Alu.add)
        ono = const.tile([128, 1], F32)
        nc.vector.tensor_scalar(smk[:], smk[:], -1.0, float(NUM), Alu.mult, Alu.add)
        nc.vector.tensor_tensor(ono[:], smk[:], oq[:], Alu.mult)
        nc.sync.dma_start(onoobj[:], ono[:])

        # ---------------- warm coords (batch 63 cells) ------------------------
        warm = const.tile([128, 2], F32)
        scr20 = const.tile([128, 20], F32)
        nc.vector.scalar_tensor_tensor(scr20[:], pp[:], 1.0, pp[:], Alu.mult,
                                       Alu.mult, accum_out=warm[:, 0:1])
        nc.vector.tensor_reduce(warm[:, 1:2], pv[:, :, 0:2], Ax.XY, Alu.add)
        nc.sync.dma_start(owarm[:], warm[:])

        # ---------------- gather: cell channels at truth cells ----------------
        cellT = const.tile([128, BL * T], F32)   # [ch, (b,t)]
        for b in range(BL):
            xt = xpool.tile([128, HW], F32, tag="xt")
            nc.sync.dma_start(xt[0:125, :], xin[b])
            nc.gpsimd.indirect_copy(cellT[:, b * T:(b + 1) * T], xt[:],
                                    gx[:, b * 4:(b + 1) * 4], True)

        # ---------------- per-(b,t) losses, 2 batches per tile ----------------
        for k in range(TILES):
            tp = psum.tile([ROWS, 128], F32, space="PSUM")
            nc.tensor.transpose(tp[:], cellT[:, k * ROWS:(k + 1) * ROWS], ident[:])
            cell = work.tile([ROWS, 128], F32, tag="cell")
            nc.vector.tensor_copy(cell[:], tp[:])
            tvk = work.tile([ROWS, 28], F32, tag="tv")
            nc.sync.dma_start(tvk[:], tv[k])
            ob = work.tile([ROWS, 3], F32, tag="ob")

            wt, ht = tvk[:, 0:1], tvk[:, 1:2]
            at, tw, th = tvk[:, 2:3], tvk[:, 3:4], tvk[:, 4:5]
            s2v, txy2, vld = tvk[:, 5:6], tvk[:, 6:7], tvk[:, 7:8]
            cv = cell[:, 0:125].rearrange("p (n c) -> p n c", c=AL)
            wv, hv = cv[:, :, 2], cv[:, :, 3]

            t1 = work.tile([ROWS, 5], F32, tag="t1")
            t2 = work.tile([ROWS, 5], F32, tag="t2")
            t3 = work.tile([ROWS, 5], F32, tag="t3")
            t4 = work.tile([ROWS, 5], F32, tag="t4")
            sc = work.tile([ROWS, 8], F32, tag="sc")  # m|cnt|wb|hb|dw|dh|q1|q2
            nc.vector.tensor_scalar(t1[:], wv, wt, 0.0, Alu.min, Alu.max)
            nc.vector.tensor_scalar(t2[:], hv, ht, 0.0, Alu.min, Alu.max)
            nc.vector.tensor_tensor(t3[:], t1[:], t2[:], Alu.mult)    # inter
            nc.vector.tensor_tensor(t4[:], wv, hv, Alu.mult)          # wp*hp
            nc.vector.scalar_tensor_tensor(t4[:], t3[:], -1.0, t4[:],
                                           Alu.mult, Alu.add)
            nc.vector.tensor_scalar(t4[:], t4[:], at, 1e-12, Alu.add, Alu.max)
            nc.vector.reciprocal(t4[:], t4[:])
            nc.vector.tensor_tensor(t3[:], t3[:], t4[:], Alu.mult)    # iou
            nc.vector.tensor_reduce(sc[:, 0:1], t3[:], Ax.X, Alu.max)
            nc.vector.tensor_scalar(t1[:], t3[:], sc[:, 0:1], None, Alu.is_ge,
                                    Alu.add, accum_out=sc[:, 1:2])
            nc.vector.reciprocal(sc[:, 1:2], sc[:, 1:2])
            nc.vector.tensor_scalar(t1[:], t1[:], sc[:, 1:2], None, Alu.mult)
            nc.vector.scalar_tensor_tensor(t2[:], t1[:], 1.0, wv, Alu.mult,
                                           Alu.mult, accum_out=sc[:, 2:3])
            nc.vector.scalar_tensor_tensor(t2[:], t1[:], 1.0, hv, Alu.mult,
                                           Alu.mult, accum_out=sc[:, 3:4])
            # coords
            nc.vector.scalar_tensor_tensor(sc[:, 4:5], sc[:, 2:3], -1.0, tw,
                                           Alu.mult, Alu.add)
            nc.vector.scalar_tensor_tensor(sc[:, 5:6], sc[:, 3:4], -1.0, th,
                                           Alu.mult, Alu.add)
            nc.vector.scalar_tensor_tensor(sc[:, 6:7], sc[:, 4:5], sc[:, 4:5],
                                           txy2, Alu.mult, Alu.add)
            nc.vector.scalar_tensor_tensor(sc[:, 7:8], sc[:, 5:6], sc[:, 5:6],
                                           sc[:, 6:7], Alu.mult, Alu.add)
            nc.vector.tensor_tensor(ob[:, 0:1], sc[:, 7:8], s2v, Alu.mult)
            # obj
            od = work.tile([ROWS, 1], F32, tag="od")
            nc.vector.tensor_scalar(od[:], cell[:, 4:5], 1.0, None, Alu.subtract)
            nc.vector.scalar_tensor_tensor(ob[:, 1:2], od[:], od[:], vld,
                                           Alu.mult, Alu.mult)
            # classes
            ca = work.tile([ROWS, CLASSES], F32, tag="ca")
            cb = work.tile([ROWS, CLASSES], F32, tag="cb")
            nc.vector.tensor_scalar(ca[:], cell[:, 5:25], t1[:, 0:1], None,
                                    Alu.mult)
            for n in range(1, NUM):
                src, dst = (ca, cb) if n % 2 == 1 else (cb, ca)
                nc.vector.scalar_tensor_tensor(
                    dst[:], cell[:, AL * n + 5:AL * n + 25], t1[:, n:n + 1],
                    src[:], Alu.mult, Alu.add)
            fin = ca if (NUM - 1) % 2 == 0 else cb
            oth = cb if fin is ca else ca
            nc.vector.tensor_tensor(oth[:], tvk[:, 8:28], fin[:], Alu.subtract)
            sq1 = work.tile([ROWS, 1], F32, tag="sq1")
            nc.vector.scalar_tensor_tensor(fin[:], oth[:], 1.0,
                                           oth[:], Alu.mult, Alu.mult,
                                           accum_out=sq1[:])
            nc.vector.tensor_tensor(ob[:, 2:3], sq1[:], vld, Alu.mult)
            nc.sync.dma_start(obt[k], ob[:])
    nc.finalize()
    return nc


def _wrap_idx(idx):
    """[n] int -> [128, ceil(n/16)] wrapped uint16 (replicated per 16-part group)."""
    n = len(idx)
    cols = -(-n // 16)
    pad = np.zeros(cols * 16, np.uint16)
    pad[:n] = idx
    blk = pad.reshape(cols, 16).T          # [16, cols]
    return np.tile(blk, (8, 1))            # [128, cols]


def _prep(x, truth, anchors):
    f32 = np.float32
    x = np.ascontiguousarray(x, f32)
    truth = np.ascontiguousarray(truth, f32)
    anchors = np.asarray(anchors, f32)

    wt, ht = truth[..., 2], truth[..., 3]
    valid = np.cumprod((wt >= 1e-5).astype(f32), axis=1, dtype=f32)
    i = np.clip((truth[..., 0] * f32(W)).astype(np.int32), 0, W - 1)
    j = np.clip((truth[..., 1] * f32(H)).astype(np.int32), 0, H - 1)
    lin = (j * W + i).astype(np.int64)
    tx = i.astype(f32) / f32(W)
    ty = j.astype(f32) / f32(H)
    tw = np.exp(wt) * anchors[2 * (NUM - 1)] / f32(W)
    th = np.exp(ht) * anchors[2 * (NUM - 1) + 1] / f32(H)
    at = wt * ht
    scale = (f32(2.0) - at).astype(f32)
    s2v = scale * scale * valid
    txy2 = tx * tx + ty * ty
    ct = np.clip(truth[..., 4].astype(np.int32), 0, CLASSES - 1)
    oh = np.eye(CLASSES, dtype=f32)[ct]                      # [B, T, 20]
    tvfull = np.stack([wt, ht, at, tw, th, s2v, txy2, valid], -1)  # [B,T,8]
    tvfull = np.concatenate([tvfull, oh], -1).astype(f32)    # [B,T,28]

    xp63 = x[B - 1].reshape(NUM * AL, HW)
    t63 = truth[B - 1]
    bl = t63[:, 0] - f32(0.5) * t63[:, 2]
    bu = t63[:, 1] - f32(0.5) * t63[:, 3]
    br = t63[:, 0] + f32(0.5) * t63[:, 2]
    bd = t63[:, 1] + f32(0.5) * t63[:, 3]
    hat = f32(0.5) * (t63[:, 2] * t63[:, 3])
    tbrow = np.concatenate([bl, bu, br, bd, hat]).astype(f32)  # [250]
    tb63 = np.tile(tbrow[None, :], (128, 1))

    in_maps = []
    for c in range(NCORES):
        bs = slice(BL * c, BL * (c + 1))
        cells = slice(CELLS * c, CELLS * (c + 1))
        gidx = np.hstack([_wrap_idx(lin[BL * c + b]) for b in range(BL)])
        p63 = np.empty((128, 20), f32)
        for n in range(NUM):
            for cc in range(4):
                p63[:, n * 4 + cc] = xp63[AL * n + cc, cells]
        o63sq = (xp63[4, cells] ** 2).astype(f32)[:, None]
        in_maps.append({
            "xin": x[bs].reshape(BL, NUM * AL, HW),
            "gidx": gidx.astype(np.uint16),
            "tv": tvfull[bs].reshape(TILES, ROWS, 28),
            "p63": p63,
            "o63sq": o63sq,
            "tb63": tb63,
        })
    return in_maps


def _combine(results):
    obj = sum(float(r["obt"][..., 1].sum(dtype=np.float64)) for r in results)
    cls = sum(float(r["obt"][..., 2].sum(dtype=np.float64)) for r in results)
    coord63 = float(results[NCORES - 1]["obt"][TILES - 1, T:ROWS, 0]
                    .sum(dtype=np.float64))
    noobj = sum(float(r["onoobj"].sum(dtype=np.float64)) for r in results)
    sq = sum(float(r["owarm"][:, 0].sum(dtype=np.float64)) for r in results)
    xy = sum(float(r["owarm"][:, 1].sum(dtype=np.float64)) for r in results)
    warm = 0.01 * (sq - xy + 0.5 * NUM * HW)   # +2560: sum of (0.5^2)*2 per (n,cell)
    return np.float32(obj + noobj + warm + coord63 + cls)


def kernel(x, truth, anchors, **_):
    if "nc" not in _CACHED:
        _CACHED["nc"] = _build_program()
    nc = _CACHED["nc"]
    in_maps = _prep(x, truth, anchors)
    res = run_bass_kernel_spmd(nc, in_maps, list(range(NCORES)))
    return _combine(res.results)



# revision 7
# speedup vs baseline: 1.2691x; 1.2691x over previous
"""YOLOv2-style loss (nn_CostYoloV2) on 8 Trainium2 NeuronCores.

Sharding:
  * per-(batch,truth) losses (obj / class / coords): data-parallel over batch,
    8 batches per core.  Only the 115 channels actually consumed downstream
    (per-anchor w,h,obj,classes = ch n*25+2..24) are DMA'd, as one affine
    3D access pattern per batch.  Channel gathers at the truth cells happen
    on-chip (gpsimd indirect_copy), then a PE transpose yields [bt, ch]
    tiles for the vector/scalar-engine loss math.
  * the noobj term and the coord warmup term only involve batch 63 - those
    are sharded over the 1024 spatial cells (128 cells per core).
  * scalar partials are returned per core and combined on the host (the
    "all-reduce" of the loss terms).

Perf structure: per-batch DMAs alternate between the two HWDGE queues
(sync + scalar engines) so transfers pipeline; gather/transpose/math are
emitted per batch-pair so the tile scheduler overlaps them with the DMA
stream; elementwise work is split between the Vector and Scalar engines.
"""
import numpy as np
from contextlib import ExitStack

import concourse.bass as bass
import concourse.bacc as bacc
import concourse.mybir as mybir
import concourse.tile as tile
from concourse.bass_utils import run_bass_kernel_spmd
from concourse.masks import make_identity

F32 = mybir.dt.float32
U16 = mybir.dt.uint16
Alu = mybir.AluOpType
Ax = mybir.AxisListType
Act = mybir.ActivationFunctionType

B, NUM, CLASSES, AL = 64, 5, 20, 25
H = W = 32
HW = H * W
T = 50
NCORES = 8
BL = B // NCORES      # local batches per core
TILES = BL // 2       # bt tiles of 2 batches x 50 truths
ROWS = 2 * T          # 100 partitions per bt tile
CELLS = HW // NCORES  # 128 noobj cells per core
CH = 23               # channels kept per anchor (25 minus unused pred x,y)
NCH = NUM * CH        # 115
ICOLS = -(-T // 16)  # 4 u16 index columns per batch (wrapped by 16)

_CACHED = {}


def _build_program():
    nc = bacc.Bacc()
    xin = nc.declare_dram_parameter("xin", [BL, NUM, CH, HW], F32, isOutput=False)
    gidx = nc.declare_dram_parameter("gidx", [128, BL * ICOLS], U16, isOutput=False)
    tv = nc.declare_dram_parameter("tv", [ROWS, TILES * 28], F32, isOutput=False)
    p63 = nc.declare_dram_parameter("p63", [128, 21], F32, isOutput=False)
    tb63 = nc.declare_dram_parameter("tb63", [128, 250], F32, isOutput=False)
    ob4 = nc.declare_dram_parameter("ob4", [ROWS, TILES * 3], F32, isOutput=True)
    onw = nc.declare_dram_parameter("onw", [128, 3], F32, isOutput=True)

    with tile.TileContext(nc) as tc, ExitStack() as ctx:
        const = ctx.enter_context(tc.tile_pool(name="const", bufs=1))
        xpool = ctx.enter_context(tc.tile_pool(name="xp", bufs=4))
        work = ctx.enter_context(tc.tile_pool(name="wk", bufs=4))
        psum = ctx.enter_context(tc.tile_pool(name="ps", bufs=4, space="PSUM"))

        ident = const.tile([128, 128], F32)
        make_identity(nc, ident[:])

        # ---------------- input DMAs (two HWDGE queues) -----------------------
        gx = const.tile([128, BL * ICOLS], U16)
        nc.sync.dma_start(gx[:], gidx[:])
        tb = const.tile([128, 250], F32)
        nc.sync.dma_start(tb[:], tb63[:])
        pp = const.tile([128, 21], F32)
        nc.scalar.dma_start(pp[:], p63[:])
        tvt = const.tile([ROWS, TILES * 28], F32)
        nc.scalar.dma_start(tvt[:], tv[:])

        xts = []
        for k in range(TILES):
            xt = xpool.tile([128, 2 * HW], F32, tag="xt", name=f"xt{k}")
            xts.append(xt)
        for b in range(BL):
            eng = nc.sync if b % 2 == 0 else nc.scalar
            xt = xts[b // 2]
            half = (b % 2) * HW
            eng.dma_start(xt[0:NCH, half:half + HW], xin[b])

        # ---------------- noobj (batch 63, this core's 128 cells) -------------
        pv = pp[:, 0:20].rearrange("p (n c) -> p n c", c=4)
        xc, yc, wc, hc = pv[:, :, 0], pv[:, :, 1], pv[:, :, 2], pv[:, :, 3]
        corn = const.tile([128, 25], F32)  # al|au|ar|ad|hap blocks of 5
        al, au = corn[:, 0:5], corn[:, 5:10]
        ar, ad = corn[:, 10:15], corn[:, 15:20]
        hap = corn[:, 20:25]
        nc.vector.scalar_tensor_tensor(al, wc, -0.5, xc, Alu.mult, Alu.add)
        nc.vector.scalar_tensor_tensor(au, hc, -0.5, yc, Alu.mult, Alu.add)
        nc.vector.scalar_tensor_tensor(ar, wc, 0.5, xc, Alu.mult, Alu.add)
        nc.vector.scalar_tensor_tensor(ad, hc, 0.5, yc, Alu.mult, Alu.add)
        nc.vector.scalar_tensor_tensor(hap, wc, 0.5, hc, Alu.mult, Alu.mult)

        def bc_p(apv):  # [128, 5] -> [128, 5, 50]
            return apv.broadcast_to([128, 5, T])

        def bc_t(col):  # tb block [128, 50] -> [128, 5, 50]
            return tb[:, col * T:(col + 1) * T].rearrange(
                "p (o f) -> p o f", o=1).broadcast_to([128, 5, T])

        def w3(tl):  # [128, 250] tile -> [128, 5, 50] view
            return tl[:].rearrange("p (n t) -> p n t", t=T)

        m1 = const.tile([128, 5 * T], F32)
        m2 = const.tile([128, 5 * T], F32)
        iw = const.tile([128, 5 * T], F32)
        ih = const.tile([128, 5 * T], F32)
        nc.vector.tensor_tensor(w3(m1), bc_t(2), bc_p(ar), Alu.min)
        nc.vector.tensor_tensor(w3(m2), bc_t(0), bc_p(al), Alu.max)
        nc.vector.tensor_tensor(iw[:], m1[:], m2[:], Alu.subtract)
        nc.scalar.activation(iw[:], iw[:], Act.Relu)
        nc.vector.tensor_tensor(w3(m1), bc_t(3), bc_p(ad), Alu.min)
        nc.vector.tensor_tensor(w3(m2), bc_t(1), bc_p(au), Alu.max)
        nc.vector.tensor_tensor(ih[:], m1[:], m2[:], Alu.subtract)
        nc.scalar.activation(ih[:], ih[:], Act.Relu)
        nc.vector.tensor_tensor(iw[:], iw[:], ih[:], Alu.mult)   # inter
        nc.scalar.mul(iw[:], iw[:], 1.5)
        nc.vector.tensor_tensor(w3(iw), w3(iw), bc_p(hap), Alu.subtract)
        nc.vector.tensor_tensor(w3(m1), w3(iw), bc_t(4), Alu.is_gt)  # mask
        anyt = const.tile([128, 6], F32)
        nc.vector.tensor_reduce(anyt[:, 0:5], w3(m1), Ax.X, Alu.max)
        smk = const.tile([128, 1], F32)
        nc.vector.tensor_reduce(smk[:], anyt[:, 0:5], Ax.X, Alu.add)
        oq2 = const.tile([128, 1], F32)
        nc.scalar.square(oq2[:], pp[:, 20:21])
        now = const.tile([128, 3], F32)
        nc.vector.tensor_scalar(smk[:], smk[:], -1.0, float(NUM), Alu.mult, Alu.add)
        nc.vector.tensor_tensor(now[:, 0:1], smk[:], oq2[:], Alu.mult)

        # ---------------- warm coords (batch 63 cells) ------------------------
        scr20 = const.tile([128, 20], F32)
        nc.scalar.activation(scr20[:], pp[:, 0:20], Act.Square,
                             accum_out=now[:, 1:2])
        nc.vector.tensor_reduce(now[:, 2:3], pv[:, :, 0:2], Ax.XY, Alu.add)
        nc.sync.dma_start(onw[:], now[:])

        # ---------------- per-(b,t) losses, one pair (2 batches) at a time ----
        ob = const.tile([ROWS, TILES * 3], F32)
        for k in range(TILES):
            cl = work.tile([128, ROWS], F32, tag="cl", name=f"cl{k}")
            for h in range(2):
                b = 2 * k + h
                nc.gpsimd.indirect_copy(cl[:, h * T:(h + 1) * T],
                                        xts[k][:, h * HW:(h + 1) * HW],
                                        gx[:, b * ICOLS:(b + 1) * ICOLS], True)
            tp = psum.tile([ROWS, 128], F32, space="PSUM", tag="tp",
                           name=f"tp{k}")
            nc.tensor.transpose(tp[:], cl[:], ident[:])
            cell = work.tile([ROWS, NCH], F32, tag="cell", name=f"cell{k}")
            nc.scalar.copy(cell[:], tp[:, 0:NCH])

            tvk = tvt[:, k * 28:(k + 1) * 28]
            wt, ht = tvk[:, 0:1], tvk[:, 1:2]
            at, tw, th = tvk[:, 2:3], tvk[:, 3:4], tvk[:, 4:5]
            s2v, txy2, vld = tvk[:, 5:6], tvk[:, 6:7], tvk[:, 7:8]
            cv = cell[:].rearrange("p (n c) -> p n c", c=CH)
            wv, hv = cv[:, :, 0], cv[:, :, 1]

            t1 = work.tile([ROWS, 5], F32, tag="t1", name=f"t1_{k}")
            t2 = work.tile([ROWS, 5], F32, tag="t2", name=f"t2_{k}")
            t3 = work.tile([ROWS, 5], F32, tag="t3", name=f"t3_{k}")
            t4 = work.tile([ROWS, 5], F32, tag="t4", name=f"t4_{k}")
            sc = work.tile([ROWS, 8], F32, tag="sc", name=f"sc{k}")
            # sc cols: 0 m | 1 wb | 2 hb | 3 sq1 | 4 q1 | 5 q2 | 6 od
            nc.vector.tensor_scalar(t1[:], wv, wt, 0.0, Alu.min, Alu.max)
            nc.vector.tensor_scalar(t2[:], hv, ht, 0.0, Alu.min, Alu.max)
            nc.vector.tensor_tensor(t3[:], t1[:], t2[:], Alu.mult)    # inter
            nc.vector.tensor_tensor(t4[:], wv, hv, Alu.mult)          # wp*hp
            nc.vector.scalar_tensor_tensor(t4[:], t3[:], -1.0, t4[:],
                                           Alu.mult, Alu.add)
            nc.vector.tensor_scalar(t4[:], t4[:], at, 1e-12, Alu.add, Alu.max)
            nc.vector.reciprocal(t4[:], t4[:])
            nc.vector.tensor_tensor(t3[:], t3[:], t4[:], Alu.mult)    # iou
            nc.vector.tensor_reduce(sc[:, 0:1], t3[:], Ax.X, Alu.max)
            nc.vector.tensor_scalar(t1[:], t3[:], sc[:, 0:1], None, Alu.is_ge)
            nc.vector.scalar_tensor_tensor(t2[:], t1[:], 1.0, wv, Alu.mult,
                                           Alu.mult, accum_out=sc[:, 1:2])
            nc.vector.scalar_tensor_tensor(t2[:], t1[:], 1.0, hv, Alu.mult,
                                           Alu.mult, accum_out=sc[:, 2:3])
            # coords (scalar engine): q1=(tw-wb)^2, q2=(th-hb)^2,
            # ob0 = (q1+q2+txy2)*s2v
            nc.scalar.activation(sc[:, 4:5], sc[:, 1:2], Act.Square,
                                 bias=tw, scale=-1.0)
            nc.scalar.activation(sc[:, 5:6], sc[:, 2:3], Act.Square,
                                 bias=th, scale=-1.0)
            nc.scalar.add(sc[:, 4:5], sc[:, 4:5], sc[:, 5:6])
            nc.scalar.add(sc[:, 4:5], sc[:, 4:5], txy2)
            nc.scalar.mul(ob[:, 3 * k:3 * k + 1], sc[:, 4:5], s2v)
            # obj (scalar engine): ob1 = (obj-1)^2 * vld
            nc.scalar.activation(sc[:, 6:7], cell[:, 2:3], Act.Square,
                                 bias=1.0, scale=-1.0)
            nc.scalar.mul(ob[:, 3 * k + 1:3 * k + 2], sc[:, 6:7], vld)
            # classes
            ca = work.tile([ROWS, CLASSES], F32, tag="ca", name=f"ca{k}")
            cb = work.tile([ROWS, CLASSES], F32, tag="cb", name=f"cb{k}")
            nc.vector.tensor_scalar(ca[:], cell[:, 3:23], t1[:, 0:1], None,
                                    Alu.mult)
            for n in range(1, NUM):
                src, dst = (ca, cb) if n % 2 == 1 else (cb, ca)
                nc.vector.scalar_tensor_tensor(
                    dst[:], cell[:, CH * n + 3:CH * n + 23], t1[:, n:n + 1],
                    src[:], Alu.mult, Alu.add)
            fin = ca if (NUM - 1) % 2 == 0 else cb
            oth = cb if fin is ca else ca
            nc.vector.tensor_tensor(oth[:], tvk[:, 8:28], fin[:], Alu.subtract)
            nc.vector.scalar_tensor_tensor(fin[:], oth[:], 1.0,
                                           oth[:], Alu.mult, Alu.mult,
                                           accum_out=sc[:, 3:4])
            nc.scalar.mul(ob[:, 3 * k + 2:3 * k + 3], sc[:, 3:4], vld)
        nc.sync.dma_start(ob4[:], ob[:])
    nc.finalize()
    return nc


def _wrap_idx(idx):
    """[n] int -> [128, ceil(n/16)] wrapped uint16 (replicated per 16-part group)."""
    n = len(idx)
    cols = -(-n // 16)
    pad = np.zeros(cols * 16, np.uint16)
    pad[:n] = idx
    blk = pad.reshape(cols, 16).T          # [16, cols]
    return np.tile(blk, (8, 1))            # [128, cols]


def _prep(x, truth, anchors):
    f32 = np.float32
    x = np.ascontiguousarray(x, f32)
    truth = np.ascontiguousarray(truth, f32)
    anchors = np.asarray(anchors, f32)

    wt, ht = truth[..., 2], truth[..., 3]
    valid = np.cumprod((wt >= 1e-5).astype(f32), axis=1, dtype=f32)
    i = np.clip((truth[..., 0] * f32(W)).astype(np.int32), 0, W - 1)
    j = np.clip((truth[..., 1] * f32(H)).astype(np.int32), 0, H - 1)
    lin = (j * W + i).astype(np.int64)
    tx = i.astype(f32) / f32(W)
    ty = j.astype(f32) / f32(H)
    tw = np.exp(wt) * anchors[2 * (NUM - 1)] / f32(W)
    th = np.exp(ht) * anchors[2 * (NUM - 1) + 1] / f32(H)
    at = wt * ht
    scale = (f32(2.0) - at).astype(f32)
    s2v = scale * scale * valid
    txy2 = tx * tx + ty * ty
    ct = np.clip(truth[..., 4].astype(np.int32), 0, CLASSES - 1)
    oh = np.eye(CLASSES, dtype=f32)[ct]                      # [B, T, 20]
    tvfull = np.stack([wt, ht, at, tw, th, s2v, txy2, valid], -1)  # [B,T,8]
    tvfull = np.concatenate([tvfull, oh], -1).astype(f32)    # [B,T,28]

    xp63 = x[B - 1].reshape(NUM * AL, HW)
    t63 = truth[B - 1]
    bl = t63[:, 0] - f32(0.5) * t63[:, 2]
    bu = t63[:, 1] - f32(0.5) * t63[:, 3]
    br = t63[:, 0] + f32(0.5) * t63[:, 2]
    bd = t63[:, 1] + f32(0.5) * t63[:, 3]
    hat = f32(0.5) * (t63[:, 2] * t63[:, 3])
    tbrow = np.concatenate([bl, bu, br, bd, hat]).astype(f32)  # [250]
    tb63 = np.tile(tbrow[None, :], (128, 1))

    x5 = x.reshape(B, NUM, AL, HW)
    in_maps = []
    for c in range(NCORES):
        bs = slice(BL * c, BL * (c + 1))
        cells = slice(CELLS * c, CELLS * (c + 1))
        gidx = np.hstack([_wrap_idx(lin[BL * c + b]) for b in range(BL)])
        p63 = np.empty((128, 21), f32)
        for n in range(NUM):
            for cc in range(4):
                p63[:, n * 4 + cc] = xp63[AL * n + cc, cells]
        p63[:, 20] = xp63[4, cells]
        tvc = tvfull[bs].reshape(TILES, 2, T, 28)
        in_maps.append({
            "xin": x5[bs, :, 2:25, :],
            "gidx": gidx.astype(np.uint16),
            "tv": np.ascontiguousarray(
                tvc.transpose(1, 2, 0, 3).reshape(ROWS, TILES * 28)),
            "p63": p63,
            "tb63": tb63,
        })
    return in_maps


def _combine(results):
    obj = sum(float(r["ob4"][:, 1::3].sum(dtype=np.float64)) for r in results)
    cls = sum(float(r["ob4"][:, 2::3].sum(dtype=np.float64)) for r in results)
    coord63 = float(results[NCORES - 1]["ob4"][T:ROWS, 3 * (TILES - 1)]
                    .sum(dtype=np.float64))
    noobj = sum(float(r["onw"][:, 0].sum(dtype=np.float64)) for r in results)
    sq = sum(float(r["onw"][:, 1].sum(dtype=np.float64)) for r in results)
    xy = sum(float(r["onw"][:, 2].sum(dtype=np.float64)) for r in results)
    warm = 0.01 * (sq - xy + 0.5 * NUM * HW)   # +2560: sum of (0.5^2)*2 per (n,cell)
    return np.float32(obj + noobj + warm + coord63 + cls)


def kernel(x, truth, anchors, **_):
    if "nc" not in _CACHED:
        _CACHED["nc"] = _build_program()
    nc = _CACHED["nc"]
    in_maps = _prep(x, truth, anchors)
    res = run_bass_kernel_spmd(nc, in_maps, list(range(NCORES)))
    return _combine(res.results)
